# revision 23
# baseline (speedup 1.0000x reference)
"""LIF spiking-neuron recurrence on Trainium2 (8 NeuronCores).

Reference semantics (TAU=1, THRESH=1, f32):
    mem = 0
    for t in range(T):
        mem = mem + x[t]
        spike[t] = (mem >= 1.0) ? 1.0 : 0.0
        mem = mem * (1 - spike[t])        # hard reset

Sharding: data-parallel over batch (B=128 -> 16 rows/core); per-core
[T, 16, 16384] viewed as [T, 128, 2048], host-transposed to [128, T, 2048].

Two tricks take this from DVE-bound (~282us for the old PE+DVE hybrid)
to the HBM roofline:

1. Fused recurrence op. A runtime-registered custom DVE op folds the
   previous step's reset into this step's add, so the recurrence is ONE
   1x DVE pass/step (2.29us measured, formula-exact) instead of TT+STT
   (3.5us):

       tmp_t = select(tmp_{t-1} < 1, tmp_{t-1}, 0) + x_t   (LIF_STEP_ANT)

   tmp_t is the pre-reset membrane; mem_t is never materialized.
   Bit-exact: select passes tmp through untouched (incl -0.0) below
   threshold and yields +0.0 at/above it, matching tmp*(1-spike) in f32.
   ACT computes mask_t = [tmp_t < 1] as u8 via Sign(1 - tmp_t) (negative
   Sign saturates to u8 0; Sign(0)=0 handles the exact tie tmp==1).
   tmp_0 is x_0 itself - the t=0 op is skipped.

2. Bit-packed output (16MiB -> 2MiB/core). The otherwise-idle PE packs
   8 steps of masks into one byte: mask u8 {0,1} BITCAST to fp8e4 reads
   as {0, 2^-9} (denormal; probed NOT flushed by the PE), matmul'd
   against stationary diag(2^(9+k)) bf16 weights with PSUM f32
   accumulation over k=0..7 -> exact integers 0..255. ACT Copy extracts
   PSUM f32 -> SBUF u8 once per 8 steps (probed exact on all 256 byte
   values; kept off the saturated DVE). The last 8 steps skip packing
   and stream raw masks (input DMA is done by then; cuts the serial
   Sign->MM->extract tail). Host unpacks bits.

Steady state is input-DMA-paced: LIF ops start every ~2.49us = x slab
arrival at ~420GB/s (the 16x27GB/s SDMA fabric ceiling per core; the
HBM stack adds a ~713GB/s cap shared by NC pairs). Engine busy/step:
DVE 2.29us, ACT 2.0 + 0.25(extract), PE ~0.9, all under the DMA pace.
Per-core DMA 84MB -> 69.5MB; best measured 192764ns (~ the 2x69.5MB /
713GB/s stack roofline; runs on a heat-soaked device throttle to
~230-240us - check summary.throttle_active_nc0_time_ns in the NTFF).

Output DMAs ride the Scalar HWDGE queue on purpose: putting them on the
Sync queue (which carries the x stream) interleaves small out-packets
into the input stream at descriptor granularity and costs ~15%.
"""

import numpy as np

try:
    import concourse  # noqa: F401
except ImportError:  # pragma: no cover
    import sys

    for _p in ("/opt/trn_rl_repo", "/root/.axon_site/_ro/trn_rl_repo"):
        if _p not in sys.path:
            sys.path.insert(0, _p)

import ml_dtypes

from concourse import bacc, mybir
from concourse.bass_utils import run_bass_kernel_spmd
from concourse.mybir import ActivationFunctionType as AF
from concourse.tile import TileContext

T, B, D = 64, 128, 16384
NCORES = 8
BL = B // NCORES  # 16 batch rows per core
P = 128
F = (BL * D) // P  # 2048
PK = 8  # timesteps packed per output byte
NG = T // PK  # output groups


def register_lif_op():
    """Runtime-register the fused LIF-step DVE op:
    out = select(in0 < s0, in0, 0) + in1  (s0 = threshold)."""
    from concourse import dve_ops
    from concourse.dve_ops import (
        OPS,
        DveOp,
        _CUSTOM_DVE_ROW_BASE,
        _SUB_OPCODE_FOR_NAME,
    )
    from concourse.dve_spec import C0, Spec, Src0, Src1, Zero, select

    if "LIF_STEP_ANT" in _SUB_OPCODE_FOR_NAME:
        return next(op for op in OPS if op.name == "LIF_STEP_ANT")

    spec = Spec(
        body=select(Src0 < C0, Src0, Zero) + Src1,
        reference=lambda in0, in1, s0, s1, imm2: (
            np.where(in0 < s0, in0, np.float32(0.0)) + in1
        ).astype(np.float32),
    )
    op = DveOp(
        "LIF_STEP_ANT",
        spec,
        subdim=False,
        uops_sha={"v3": "38f6b55dbeb193f6", "v4": "cb4fb9e0c41a0972"},
    )
    OPS.append(op)
    _SUB_OPCODE_FOR_NAME[op.name] = _CUSTOM_DVE_ROW_BASE + len(OPS) - 1
    dve_ops.CUSTOM_DVE_SPECS[op.name] = op.spec
    return op


def build_nc(t_steps=T, x_chunk=4, x_bufs=4):
    assert t_steps % PK == 0
    f32 = mybir.dt.float32
    u8 = mybir.dt.uint8
    bf16 = mybir.dt.bfloat16
    fp8e4 = mybir.dt.float8e4
    lif = register_lif_op()
    nc = bacc.Bacc(
        "TRN2", target_bir_lowering=False, debug=False, num_devices=NCORES
    )
    x_ext = nc.dram_tensor("x", [P, t_steps, F], f32, kind="ExternalInput")
    w_ext = nc.dram_tensor("w", [PK, P, P], bf16, kind="ExternalInput")
    n_groups_out = t_steps // PK
    out_ext = nc.dram_tensor(
        "out", [P, n_groups_out - 1, F], u8, kind="ExternalOutput"
    )
    tail_ext = nc.dram_tensor(
        "tail", [P, PK, F], u8, kind="ExternalOutput"
    )
    n_xgroups = (t_steps + x_chunk - 1) // x_chunk
    with TileContext(nc) as tc:
        with (
            tc.tile_pool(name="mp", bufs=1) as mp,
            tc.tile_pool(name="tp", bufs=4) as tp,
            tc.tile_pool(name="kp", bufs=4) as kp,
            tc.tile_pool(name="xp", bufs=x_bufs) as xp,
            tc.tile_pool(name="op", bufs=2) as op_pool,
            tc.tile_pool(name="tl", bufs=8) as tl_pool,
            tc.psum_pool(name="pp", bufs=2) as pp,
        ):
            b1 = mp.tile([P, 1], f32, name="b1")
            nc.vector.memset(b1[:], 1.0)
            ws = [mp.tile([P, P], bf16, name=f"w{k}") for k in range(PK)]

            x_tiles = {}

            def x_slab(t):
                g, j = divmod(t, x_chunk)
                return x_tiles[g][:, j * F : (j + 1) * F]

            def ensure_x_loaded(g):
                if g in x_tiles or g >= n_xgroups:
                    return
                xt = xp.tile([P, x_chunk * F], f32, name="xt")
                x_tiles[g] = xt
                xv = x_ext[:, g * x_chunk : (g + 1) * x_chunk, :]
                for j in range(x_chunk):
                    nc.sync.dma_start(
                        xt[:, j * F : (j + 1) * F], xv[:, j, :]
                    )

            ensure_x_loaded(0)
            ensure_x_loaded(1)
            # weight loads go on the scalar queue AFTER the first x groups:
            # 8 sync-queue triggers ahead of the x stream cost ~5us of ramp
            for k in range(PK):
                nc.scalar.dma_start(ws[k][:], w_ext[k, :, :])
            prev = None  # tmp_0 is x_0 itself; the first LIF op is skipped
            for og in range(n_groups_out):
                last = og == n_groups_out - 1
                ps = None if last else pp.tile([P, F], f32, name="ps")
                for k in range(PK):
                    t = og * PK + k
                    xg = t // x_chunk
                    ensure_x_loaded(xg)
                    ensure_x_loaded(xg + 1)
                    ensure_x_loaded(xg + 2)
                    if t == 0:
                        cur = x_slab(0)
                    else:
                        tmp = tp.tile([P, F], f32, name="tmp")
                        # tmp_t = select(tmp_{t-1} < 1, tmp_{t-1}, 0) + x_t
                        nc.vector._custom_dve(
                            lif, out=tmp[:], in0=prev, in1=x_slab(t), s0=1.0
                        )
                        cur = tmp[:]
                    # mask_t = [tmp_t < 1] as u8 (Sign(1-tmp); <0 saturates)
                    mk = (tl_pool if last else kp).tile([P, F], u8, name="mk")
                    nc.scalar.activation(
                        mk[:], cur, AF.Sign, bias=b1[:], scale=-1.0
                    )
                    if last:
                        # final group: stream raw masks out instead of packing
                        # (cuts the serial Sign->MM->extract->DMA tail; input
                        # DMA is finished by now so the bytes are free)
                        nc.sync.dma_start(tail_ext[:, k, :], mk[:])
                    else:
                        # pack: ps += 2^(9+k) * fp8e4(mask)  (denorm trick);
                        # one MM per 512-col PSUM bank
                        for c in range(F // 512):
                            sl = slice(c * 512, (c + 1) * 512)
                            nc.tensor.matmul(
                                ps[:, sl],
                                ws[k][:],
                                mk[:, sl].bitcast(fp8e4),
                                start=(k == 0),
                                stop=(k == PK - 1),
                            )
                    prev = cur
                if not last:
                    # extract packed byte: f32 ints 0..255 -> u8 (on ACT; DVE
                    # is saturated by the LIF ops, ACT has slack — probed exact)
                    pk_t = op_pool.tile([P, F], u8, name="pk")
                    nc.scalar.activation(pk_t[:], ps[:], AF.Copy)
                    nc.scalar.dma_start(out_ext[:, og, :], pk_t[:])
    nc.compile()
    return nc


_cached_nc = None


def _get_nc():
    global _cached_nc
    if _cached_nc is None:
        _cached_nc = build_nc()
    return _cached_nc


def _pack_weights():
    w = np.zeros((PK, P, P), dtype=ml_dtypes.bfloat16)
    for k in range(PK):
        np.fill_diagonal(w[k], np.float32(2.0 ** (9 + k)))
    return w


_W = _pack_weights()


def _shard(x):
    in_maps = []
    for c in range(NCORES):
        xc = x[:, c * BL : (c + 1) * BL, :].reshape(T, P, F).transpose(1, 0, 2)
        in_maps.append({"x": np.ascontiguousarray(xc), "w": _W})
    return in_maps


def _gather(results):
    shifts = np.arange(PK, dtype=np.uint8)[None, None, :, None]
    outs = []
    for c in range(NCORES):
        pk = np.asarray(results[c]["out"])  # [P, NG-1, F] packed mask bits
        bits = (pk[:, :, None, :] >> shifts) & 1  # [P, NG-1, PK, F]
        tail = np.asarray(results[c]["tail"])  # [P, PK, F] raw mask
        mask = np.concatenate([bits.reshape(P, T - PK, F), tail], axis=1)
        spikes = (1 - mask).astype(np.float32)
        outs.append(spikes.transpose(1, 0, 2).reshape(T, BL, D))
    return np.concatenate(outs, axis=1)


def run(x, trace=False, **kw):
    x = np.ascontiguousarray(np.asarray(x, dtype=np.float32))
    assert x.shape == (T, B, D), x.shape
    nc = _get_nc()
    res = run_bass_kernel_spmd(
        nc, _shard(x), core_ids=list(range(NCORES)), trace=trace, **kw
    )
    return _gather(res.results), res


def kernel(x: np.ndarray) -> np.ndarray:
    out, _ = run(x)
    return out
